# revision 58
# baseline (speedup 1.0000x reference)
"""MHA kernel for 8 Trainium2 NeuronCores — fp8 DoubleRow edition.

Reference computation (per batch b):
    Qh = (q[b] @ Wq.T) * Dh^-0.5, Kh = k[b] @ Wk.T, Vh = v[b] @ Wv.T   (16 heads of 128)
    P  = softmax(Qh Kh^T), O = P Vh, out[b] = concat_heads(O) @ Wo.T
Mask is all-False (spec fill=zeros) and is ignored.

Sharding: 8 cores = 2 batches x 4 head-groups (4 heads / core).
Wq/Wk/Wv split column-wise, Wo row-wise; the post-projection all-reduce is the
host-side sum of the 4 per-head-group partial outputs per batch.

Per-core kernel:
  Projections and the output projection run on the PE in fp8e4m3 with
  MatmulPerfMode.DoubleRow (two 128-deep k-planes per instruction, 0.5
  cycles/row).  Accuracy is kept at ~bf16 level with a 3-term hi/lo split:
      x @ W ~= x_hi@W_hi + x_lo@W_hi + x_hi@W_lo
  where x_hi = fp8(SX*x), x_lo = fp8(SX*x - x_hi) (same PSUM scale for all
  three terms).  Scales: SX=4 for activations, SW=128 for weights; descales
  are folded into the exp() activation scale, the softmax-denominator
  broadcast constant, and the final output drain scale.

  Attention (bf16): scoresT[m] = KhT_m^T @ QhT per 1024-col chunk ->
  exp on ACT -> P^T tiles; O^T += Vh_m^T @ P^T_m in PSUM.  Softmax
  denominator: DVE pair-sum tree (16->8->4 tiles), one-shot ones-matmuls
  into a transient PSUM tile, accumulated in SBUF (bf16) by DVE.  The
  normalize produces oT in fp8 hi/lo directly for the DoubleRow out-proj.
"""

import numpy as np
import ml_dtypes

E4 = ml_dtypes.float8_e4m3
BF16 = ml_dtypes.bfloat16

B = 2
S = 2048
D = 2048
NH_TOT = 16
DH = 128
H = 4            # heads per core
HS = H * DH      # 512, model-dim slice per core
P = 128
KD = D // P      # 16 contraction tiles over model dim
KP = KD // 2     # 8 k-pair (DoubleRow plane-pair) tiles
MT = S // P      # 16 seq tiles
N4 = S // 512    # 4 column groups of 512

SX = 4.0         # activation fp8 pre-scale
SW = 128.0       # weight fp8 pre-scale
CO = 16.0        # oT storage scale (oT tiles hold CO * O)
ES = DH ** -0.5 / (SX * SW) ** 2     # exp() input scale
BC = SX * SW / CO                    # broadcast const for 1/denom
OS = 1.0 / (CO * SW)                 # final out drain scale

_CACHE: dict = {}


def _build_bass():
    import concourse.tile as tile
    from concourse import bacc, mybir

    f32 = mybir.dt.float32
    bf16 = mybir.dt.bfloat16
    fp8 = mybir.dt.float8e4
    Exp = mybir.ActivationFunctionType.Exp
    DR = mybir.MatmulPerfMode.DoubleRow

    nc = bacc.Bacc()

    xp = {}
    for t in ("q", "k", "v"):
        for part in ("hi", "lo"):
            xp[t, part] = nc.declare_dram_parameter(
                f"x{t}_{part}", [D, S], fp8, isOutput=False)
    wp = {}
    for t in ("q", "k", "v"):
        for part in ("hi", "lo"):
            wp[t, part] = nc.declare_dram_parameter(
                f"w{t}_{part}", [D, HS], fp8, isOutput=False)
    wo_hi = nc.declare_dram_parameter("wo_hi", [HS, D], fp8, isOutput=False)
    wo_lo = nc.declare_dram_parameter("wo_lo", [HS, D], fp8, isOutput=False)
    out = nc.declare_dram_parameter("out", [S, D], bf16, isOutput=True)

    dma = nc.default_dma_engine

    with tile.TileContext(nc) as tc:
        with (
            tc.sbuf_pool(name="const", bufs=1) as cpool,
            tc.sbuf_pool(name="persist", bufs=1) as ppool,
            tc.sbuf_pool(name="small", bufs=2) as spool,
            tc.sbuf_pool(name="ostage", bufs=8) as opool,
        ):
            ones = cpool.tile([P, P], bf16, tag="ones")
            nc.vector.memset(ones, 1.0)
            bc = cpool.tile([P, P], bf16, tag="bc")
            nc.vector.memset(bc, BC)

            qhT = ppool.tile([P, H, S], bf16, tag="qhT")   # SX*SW*Qh^T
            khT = ppool.tile([P, H, S], bf16, tag="khT")   # SX*SW*Kh^T
            vh = ppool.tile([P, MT, HS], bf16, tag="vh")   # SX*SW*Vh (seq-major)
            oT_hi = ppool.tile([P, H, S], fp8, tag="oT_hi")  # fp8(CO*O^T)
            oT_lo = ppool.tile([P, H, S], fp8, tag="oT_lo")

            # ---------------- projections (fp8 DoubleRow) ----------------
            with (
                tc.sbuf_pool(name="wqkv", bufs=2) as wpool,
                tc.sbuf_pool(name="xs", bufs=24) as xpool,
                tc.psum_pool(name="pproj", bufs=8) as pjp,
            ):
                def load_w_slice(t, part, w, k0, k1):
                    dma.dma_start(
                        w[:, k0:k1, :],
                        wp[t, part][k0 * P:k1 * P, :]
                        .rearrange("(k p) n -> p k n", p=P))

                def load_w_one(t, part, half=None):
                    w = wpool.tile([P, KD, HS], fp8, tag=f"w_{part}",
                                   name=f"w{t}_{part}")
                    if half is None:
                        dma.dma_start(w, wp[t, part]
                                      .rearrange("(k p) n -> p k n", p=P))
                    else:
                        load_w_slice(t, part, w, 0, KD // 2)
                    return w

                def load_w_half2(t, part, w):
                    load_w_slice(t, part, w, KD // 2, KD)

                def load_x_stream(t, nh, w_lo_cb=None, w_hi_cb2=None,
                                  split_lo=False):
                    """DMA order: xh0, xh1, [w_lo], xh2..xh7, xl0..xl7 so the
                    A-term matmuls can start after the first hi pair.  With
                    split_lo (first tensor), w_lo streams as two halves late
                    in the x_hi stream so A-term tiles aren't delayed."""
                    xh, xl = [], []
                    w_lo = None
                    for kp in range(KP):
                        xt = xpool.tile([P, 2, 1024], fp8, tag="xt")
                        dma.dma_start(
                            xt,
                            xp[t, "hi"][kp * 256:(kp + 1) * 256,
                                        nh * 1024:(nh + 1) * 1024]
                            .rearrange("(two p) n -> p two n", p=P),
                        )
                        xh.append(xt)
                        if kp == 0 and w_hi_cb2 is not None:
                            w_hi_cb2()
                        if w_lo_cb is not None:
                            if split_lo:
                                if kp == 4:
                                    w_lo = w_lo_cb(half=0)
                                elif kp == 6:
                                    load_w_half2(t, "lo", w_lo)
                            elif kp == 1:
                                w_lo = w_lo_cb()
                    for kp in range(KP):
                        xt = xpool.tile([P, 2, 1024], fp8, tag="xt")
                        dma.dma_start(
                            xt,
                            xp[t, "lo"][kp * 256:(kp + 1) * 256,
                                        nh * 1024:(nh + 1) * 1024]
                            .rearrange("(two p) n -> p two n", p=P),
                        )
                        xl.append(xt)
                    return xh, xl, w_lo

                def proj_half(xh, xl, w_hi, w_lo, lhs_of, rhs_of, drains):
                    """Term-major 3-term fp8 projection for one S-half.
                    lhs_of/rhs_of(kp, i, xs, ws) give the DoubleRow operands
                    for output tile i; drains[i]() drains psum i."""
                    pss = [pjp.tile([P, 512], f32, tag="psproj", name="psproj")
                           for _ in range(8)]

                    def mm(kp, i, xs, ws, start, stop):
                        nc.tensor.matmul(pss[i], lhsT=lhs_of(kp, i, xs, ws),
                                         rhs=rhs_of(kp, i, xs, ws),
                                         start=start, stop=stop, perf_mode=DR)

                    for kp in range(KP - 1):            # A: x_hi (.) w_hi
                        for i in range(8):
                            mm(kp, i, xh, w_hi, kp == 0, False)
                    for kp in range(KP - 1):            # B: x_hi (.) w_lo
                        for i in range(8):
                            mm(kp, i, xh, w_lo, False, False)
                    for kp in range(KP - 1):            # C: x_lo (.) w_hi
                        for i in range(8):
                            mm(kp, i, xl, w_hi, False, False)
                    # final k-pair staggered with drains so PSUM frees early
                    for i in range(8):
                        mm(KP - 1, i, xh, w_hi, False, False)
                        mm(KP - 1, i, xh, w_lo, False, False)
                        mm(KP - 1, i, xl, w_hi, False, True)
                        drains[i](pss[i])

                def proj_qk(t, out_sb, first):
                    w_hi = load_w_one(t, "hi", half=0 if first else None)
                    w_lo = None

                    def qk_lhs(kp, i, xs, ws):
                        h = i // 2
                        return ws[:, 2 * kp:2 * kp + 2, h * P:(h + 1) * P]

                    def qk_rhs(kp, i, xs, ws):
                        n = i % 2
                        return xs[kp][:, :, n * 512:(n + 1) * 512]

                    for nh in range(2):
                        cb = ((lambda half=None: load_w_one(t, "lo", half=half))
                              if nh == 0 else None)
                        cb2 = ((lambda: load_w_half2(t, "hi", w_hi))
                               if (first and nh == 0) else None)
                        xh, xl, got = load_x_stream(t, nh, cb, cb2,
                                                    split_lo=first)
                        if got is not None:
                            w_lo = got

                        def mk_drain(i):
                            h, n = i // 2, i % 2
                            dst = out_sb[:, h, nh * 1024 + n * 512:
                                         nh * 1024 + (n + 1) * 512]
                            if i % 2 == 0:
                                return lambda ps: nc.scalar.copy(dst, ps)
                            return lambda ps: nc.vector.tensor_copy(dst, ps)

                        proj_half(xh, xl, w_hi,
                                  w_lo,
                                  lambda kp, i, xs, ws: qk_lhs(kp, i, xs, ws),
                                  lambda kp, i, xs, ws: qk_rhs(kp, i, xs, ws),
                                  [mk_drain(i) for i in range(8)])

                def proj_v():
                    w_hi = load_w_one("v", "hi")
                    w_lo = None

                    def v_lhs(kp, i, xs, ws):
                        return xs[kp][:, :, i * P:(i + 1) * P]

                    for nh in range(2):
                        cb = ((lambda half=None: load_w_one("v", "lo"))
                              if nh == 0 else None)
                        xh, xl, got = load_x_stream("v", nh, cb)
                        if got is not None:
                            w_lo = got

                        def mk_drain(i):
                            m = nh * 8 + i
                            if i % 2 == 0:
                                return lambda ps: nc.scalar.copy(vh[:, m, :], ps)
                            return lambda ps: nc.vector.tensor_copy(vh[:, m, :], ps)

                        # for V the x side is stationary (lhsT), w side moving;
                        # terms: A = x_hi.w_hi, B = x_hi.w_lo, C = x_lo.w_hi.
                        pss = [pjp.tile([P, 512], f32, tag="psproj",
                                        name="psproj") for _ in range(8)]

                        def mmv(kp, i, xs, w, start, stop):
                            nc.tensor.matmul(
                                pss[i], lhsT=v_lhs(kp, i, xs, None),
                                rhs=w[:, 2 * kp:2 * kp + 2, :],
                                start=start, stop=stop, perf_mode=DR)

                        for kp in range(KP - 1):
                            for i in range(8):
                                mmv(kp, i, xh, w_hi, kp == 0, False)
                        for kp in range(KP - 1):
                            for i in range(8):
                                mmv(kp, i, xh, w_lo, False, False)
                        for kp in range(KP - 1):
                            for i in range(8):
                                mmv(kp, i, xl, w_hi, False, False)
                        drains = [mk_drain(i) for i in range(8)]
                        for i in range(8):
                            mmv(KP - 1, i, xh, w_hi, False, False)
                            mmv(KP - 1, i, xh, w_lo, False, False)
                            mmv(KP - 1, i, xl, w_hi, False, True)
                            drains[i](pss[i])

                def warm(n_mm):
                    # harmless matmuls into psproj-rotation tiles; nobody
                    # reads them, they just keep the PE busy/clocked
                    for g in range((n_mm + 5) // 6):
                        wps = pjp.tile([P, 512], f32, tag="psproj",
                                       name="psproj")
                        for _ in range(min(6, n_mm - 6 * g)):
                            nc.tensor.matmul(wps[0:1, 0:128],
                                             lhsT=ones[:, 0:1], rhs=ones)

                warm(44)
                proj_qk("q", qhT, True)
                proj_qk("k", khT, False)
                proj_v()
                warm(10)

            # wo arrives during attention
            wo_hi_sb = ppool.tile([P, H, D], fp8, tag="wo_hi_sb")
            wo_lo_sb = ppool.tile([P, H, D], fp8, tag="wo_lo_sb")
            dma.dma_start(wo_hi_sb, wo_hi.rearrange("(k p) n -> p k n", p=P))
            dma.dma_start(wo_lo_sb, wo_lo.rearrange("(k p) n -> p k n", p=P))

            # ---------------- attention (bf16, per head) ----------------
            with (
                tc.sbuf_pool(name="pts", bufs=10) as ptpool,
                tc.sbuf_pool(name="s1s", bufs=8) as s1pool,
                tc.psum_pool(name="pattn", bufs=1) as pap,
            ):
                # denominator n-block sums live at (row, col-range):
                # n=0 -> (0, 0:512), n=1 -> (32, 0:512), n=2 -> (64, 0:512),
                # n=3 -> (0, 512:1024)  [row 96 rejected by base_partition]
                DPOS = [(0, 0), (32, 0), (64, 0), (0, 512)]

                # Fully-flat software pipeline over global steps: head h's
                # scores/exp stream occupies steps 16h+m with NO inter-head
                # bubble; PV trails at +6, the DVE pair-sum tree (into
                # separate s1 tiles, so it never blocks PV) runs right after
                # exp and condenses 16 tiles down to ONE (L1..L4); a single
                # one-shot denominator matmul group then feeds d_sb.  ps_o is
                # drained early to SBUF (oU, on the Pool engine) so the
                # normalize is fully out-of-line and next head's PV is only
                # gated on the cheap drains.  The last head's normalize is
                # interleaved with the output projection, whose PSUM tiles
                # reuse the freed ps_o slots.
                from collections import defaultdict
                sched = defaultdict(list)

                state = {}   # per-head tiles
                for h in range(H):
                    g0 = 16 * h
                    st = state[h] = {}
                    st["pt"] = {}
                    st["s1"] = {}
                    st["oU"] = {}

                    def alloc_head(h=h, st=st):
                        st["ps_o"] = [pap.tile([P, 512], f32, tag=f"ps_o{n}",
                                               bufs=1, name=f"ps_o{n}")
                                      for n in range(N4)]
                        st["d_sb"] = spool.tile([P, 1024], bf16, tag="d_sb",
                                                bufs=4, name="d_sb")
                    sched[g0].append((0.5, alloc_head))

                    def scores(h, m, c, st=st):
                        def fn():
                            if c == 0:
                                st["pt"][m] = ptpool.tile([P, S], bf16,
                                                          tag="pt", name="pt")
                            ps_s = pap.tile([P, 1024], f32, tag="ps_s", bufs=2, name="ps_s")
                            for q in range(2):
                                nc.tensor.matmul(
                                    ps_s[:, q * 512:(q + 1) * 512],
                                    lhsT=khT[:, h, m * P:(m + 1) * P],
                                    rhs=qhT[:, h, c * 1024 + q * 512:
                                            c * 1024 + (q + 1) * 512])
                            nc.scalar.activation(
                                st["pt"][m][:, c * 1024:(c + 1) * 1024],
                                ps_s, Exp, scale=ES)
                        return fn

                    def pv(h, m, st=st):
                        def fn():
                            for n in range(N4):
                                nc.tensor.matmul(
                                    st["ps_o"][n],
                                    lhsT=vh[:, m, h * P:(h + 1) * P],
                                    rhs=st["pt"][m][:, n * 512:(n + 1) * 512],
                                    start=(m == 0),
                                    stop=(m == MT - 1),
                                )
                        return fn

                    def l1(j, st=st):
                        def fn():
                            s = st["s1"][j] = s1pool.tile([P, S], bf16,
                                                          tag="s1", name="s1")
                            nc.vector.tensor_add(s, st["pt"][2 * j],
                                                 st["pt"][2 * j + 1])
                        return fn

                    def lx(dst, src, st=st):
                        def fn():
                            nc.vector.tensor_add(st["s1"][dst], st["s1"][dst],
                                                 st["s1"][src])
                        return fn

                    def denom(st=st):
                        def fn():
                            ps_t = pap.tile([P, 1024], f32, tag="ps_s", bufs=2, name="ps_s")
                            for n in range(N4):
                                r, c = DPOS[n]
                                nc.tensor.matmul(
                                    ps_t[r:r + 1, c:c + 512],
                                    lhsT=ones[:, 0:1],
                                    rhs=st["s1"][0][:, n * 512:(n + 1) * 512],
                                    tile_position=(0, r),
                                )
                            nc.vector.tensor_copy(st["d_sb"], ps_t)
                        return fn

                    def drain_o(st=st, on_act=False):
                        def fn():
                            for n in range(N4):
                                oU = st["oU"][n] = spool.tile(
                                    [P, 512], bf16, tag="oU", bufs=16, name="oU")
                                if on_act:
                                    nc.scalar.copy(oU, st["ps_o"][n])
                                else:
                                    nc.vector.tensor_copy(oU, st["ps_o"][n])
                        return fn

                    last = h == H - 1
                    for m in range(MT):
                        # first tile of a head issues one step early so the
                        # exp stream has no bubble at the head transition
                        sc_step = g0 + m - (1 if (h and m == 0) else 0)
                        sched[sc_step].append((1, scores(h, m, 0)))
                        # the last head's PV stream is compressed (2/step from
                        # m=4) so its tail doesn't delay the output projection
                        pvs = (m + 6) if not last else (
                            m + 6 if m < 4 else max(m + 1, 9 + (m - 4) // 2))
                        sched[g0 + pvs].append((2, pv(h, m)))
                        sched[sc_step].append((3, scores(h, m, 1)))
                    for j in range(8):
                        sched[g0 + 2 * j + 2].append((4, l1(j)))
                    for i in range(4):
                        sched[g0 + 4 * i + 6].append((5, lx(2 * i, 2 * i + 1)))
                    sched[g0 + 11].append((6, lx(0, 2)))

                    def denom_multi(tiles, first, st=st):
                        # one PSUM accumulation group summing several tiles
                        def fn():
                            ps_t = pap.tile([P, 1024], f32, tag="ps_s",
                                            bufs=2, name="ps_s")
                            nt = len(tiles)
                            for ti, t in enumerate(tiles):
                                src_t = st["s1"][t] if isinstance(t, int) \
                                    else st["pt"][t[1]]
                                for n in range(N4):
                                    r, c = DPOS[n]
                                    nc.tensor.matmul(
                                        ps_t[r:r + 1, c:c + 512],
                                        lhsT=ones[:, 0:1],
                                        rhs=src_t[:, n * 512:(n + 1) * 512],
                                        start=(ti == 0), stop=(ti == nt - 1),
                                        tile_position=(0, r),
                                    )
                            if first:
                                nc.vector.tensor_copy(st["d_sb"], ps_t)
                            else:
                                nc.vector.tensor_add(st["d_sb"], st["d_sb"], ps_t)
                        return fn

                    if not last:
                        sched[g0 + 19].append((6, lx(4, 6)))
                        sched[g0 + 21].append((6, lx(0, 4)))
                        sched[g0 + 25].append((7, denom()))
                        sched[g0 + 22].append((0, drain_o()))
                    else:
                        # shortest possible endgame chain for the last head
                        sched[g0 + 12].append((7, denom_multi([0], True)))
                        sched[g0 + 17].append((7, denom_multi([4, 6, 7], False)))
                        sched[g0 + 17].append((0, drain_o(on_act=True)))

                def norm_pair(h, p, ps_b_alloc, fast=False):
                    # normalize n-blocks 2p and 2p+1 of head h with one
                    # [P,1024] broadcast PSUM tile and one reciprocal.
                    # fast=True routes hi/lo to ACT+DVE (for the critical
                    # last-head normalize at outproj start).
                    st = state[h]
                    ps_b = ps_b_alloc()
                    for q in range(2):
                        n = 2 * p + q
                        r, c = DPOS[n]
                        nc.tensor.matmul(
                            ps_b[:, q * 512:(q + 1) * 512], lhsT=bc[r:r + 1, :],
                            rhs=st["d_sb"][r:r + 1, c:c + 512])
                    rb = spool.tile([P, 1024], f32, tag="rb", name="rb")
                    nc.vector.reciprocal(rb, ps_b)
                    for q in range(2):
                        n = 2 * p + q
                        t_bf = spool.tile([P, 512], bf16, tag="t_bf",
                                          bufs=4, name="t_bf")
                        nc.vector.tensor_mul(t_bf, st["oU"][n],
                                             rb[:, q * 512:(q + 1) * 512])
                        dst_hi = oT_hi[:, h, n * 512:(n + 1) * 512]
                        dst_lo = oT_lo[:, h, n * 512:(n + 1) * 512]
                        if fast:
                            nc.scalar.copy(dst_hi, t_bf)
                            nc.vector.tensor_sub(dst_lo, t_bf, dst_hi)
                        else:
                            nc.gpsimd.tensor_copy(dst_hi, t_bf)
                            nc.gpsimd.tensor_sub(dst_lo, t_bf, dst_hi)

                # each head's normalize rides early in the NEXT head's score
                # stream (Pool does the fp8 hi/lo there); h3's own normalize
                # runs block-wise at outproj start on the then-idle ACT/DVE.
                for hh in range(H - 1):
                    for pp in range(2):
                        sched[16 * (hh + 1) + 11 + 2 * pp].append(
                            (3.7, (lambda hh=hh, pp=pp: norm_pair(
                                hh, pp,
                                lambda: pap.tile([P, 1024], f32, tag="ps_s",
                                                 bufs=2, name="ps_s")))))

                for step in sorted(sched):
                    for _, fn in sorted(sched[step], key=lambda x: x[0]):
                        fn()

                # ---- output projection (fp8 DoubleRow), interleaved with the
                # last head's normalize; psf PSUM tiles reuse the ps_o slots.
                def psf_alloc(n):
                    return pap.tile([P, 512], f32, tag=f"ps_o{n}", bufs=1,
                                    name=f"ps_o{n}")

                def op_pair(psf, m, n, kh, start, stop):
                    ohi = oT_hi[:, 2 * kh:2 * kh + 2, m * P:(m + 1) * P]
                    olo = oT_lo[:, 2 * kh:2 * kh + 2, m * P:(m + 1) * P]
                    whi = wo_hi_sb[:, 2 * kh:2 * kh + 2, n * 512:(n + 1) * 512]
                    wlo = wo_lo_sb[:, 2 * kh:2 * kh + 2, n * 512:(n + 1) * 512]
                    nc.tensor.matmul(psf, lhsT=ohi, rhs=whi,
                                     start=start, stop=False, perf_mode=DR)
                    nc.tensor.matmul(psf, lhsT=olo, rhs=whi,
                                     start=False, stop=False, perf_mode=DR)
                    nc.tensor.matmul(psf, lhsT=ohi, rhs=wlo,
                                     start=False, stop=stop, perf_mode=DR)

                def op_finish(psf, m, n):
                    op_pair(psf, m, n, 1, False, True)
                    ob = opool.tile([P, 512], bf16, tag="ob")
                    if n % 2 == 0:
                        nc.scalar.mul(ob, psf, OS)
                    else:
                        nc.vector.tensor_scalar_mul(ob, psf, OS)
                    dma.dma_start(
                        out[m * P:(m + 1) * P, n * 512:(n + 1) * 512], ob)

                def norm3_block(nb):
                    # single n-block normalize for head 3, shortest latency:
                    # per-512 reciprocal, hi on ACT, lo on DVE
                    st = state[3]
                    r, c = DPOS[nb]
                    ps_b = pap.tile([P, 1024], f32, tag="ps_s", bufs=2,
                                    name="ps_s")
                    nc.tensor.matmul(ps_b[:, 0:512], lhsT=bc[r:r + 1, :],
                                     rhs=st["d_sb"][r:r + 1, c:c + 512])
                    rb = spool.tile([P, 1024], f32, tag="rb", name="rb")
                    nc.vector.reciprocal(rb[:, 0:512], ps_b[:, 0:512])
                    t_bf = spool.tile([P, 512], bf16, tag="t_bf", bufs=4,
                                      name="t_bf")
                    nc.vector.tensor_mul(t_bf, st["oU"][nb], rb[:, 0:512])
                    dst_hi = oT_hi[:, 3, nb * 512:(nb + 1) * 512]
                    nc.scalar.copy(dst_hi, t_bf)
                    nc.vector.tensor_sub(
                        oT_lo[:, 3, nb * 512:(nb + 1) * 512], t_bf, dst_hi)

                # m0: emit the (h0,h1) half-contraction first — independent of
                # head 3, it keeps the PE busy while the last head's
                # denominator/normalize chain drains on DVE/ACT.
                psf01 = {n: psf_alloc(n) for n in range(N4)}
                for n in range(N4):
                    op_pair(psf01[n], 0, n, 0, True, False)
                norm3_block(0)
                for n in range(N4):
                    op_finish(psf01[n], 0, n)
                for m in range(1, MT):
                    if m % 4 == 1 and m < 13:
                        norm3_block(m // 4 + 1)
                    if m % 2 == 0:
                        for n in range(N4):
                            psf = psf_alloc(n)
                            op_pair(psf, m, n, 0, True, False)
                            op_finish(psf, m, n)
                    else:
                        halves = []
                        for _ in range(2):
                            t = pap.tile([P, 1024], f32, tag="ps_s", bufs=2,
                                         name="ps_s")
                            halves += [t[:, 0:512], t[:, 512:1024]]
                        for n in range(N4):
                            op_pair(halves[n], m, n, 0, True, False)
                            op_finish(halves[n], m, n)

    nc.compile()
    return nc


def _get_nc():
    if "nc" not in _CACHE:
        _CACHE["nc"] = _build_bass()
    return _CACHE["nc"]


def _split8(a, scale):
    """fp8 hi/lo split of a*scale (same PSUM scale for both parts)."""
    s = np.asarray(a, np.float32) * scale
    hi = s.astype(E4)
    lo = (s - np.asarray(hi, np.float32)).astype(E4)
    return hi, lo


def _prep_inputs(q, k, v, Wq, Wk, Wv, Wo):
    """Host-side sharding: per-core transposed fp8 hi/lo slices."""
    q = np.asarray(q, np.float32)
    k = np.asarray(k, np.float32)
    v = np.asarray(v, np.float32)
    Wq = np.asarray(Wq, np.float32)
    Wk = np.asarray(Wk, np.float32)
    Wv = np.asarray(Wv, np.float32)
    Wo = np.asarray(Wo, np.float32)
    xT = {}
    for b in range(B):
        xT[b] = {}
        for t, x in (("q", q), ("k", k), ("v", v)):
            hi, lo = _split8(np.ascontiguousarray(x[b].T), SX)
            xT[b][f"x{t}_hi"] = hi
            xT[b][f"x{t}_lo"] = lo
    in_maps = []
    for c in range(8):
        b, hg = divmod(c, 4)
        hs = hg * HS
        m = dict(xT[b])
        for t, W in (("q", Wq), ("k", Wk), ("v", Wv)):
            hi, lo = _split8(np.ascontiguousarray(W[hs:hs + HS, :].T), SW)
            m[f"w{t}_hi"] = hi
            m[f"w{t}_lo"] = lo
        hi, lo = _split8(np.ascontiguousarray(Wo[:, hs:hs + HS].T), SW)
        m["wo_hi"] = hi
        m["wo_lo"] = lo
        in_maps.append(m)
    return in_maps


def run_spmd(q, k, v, Wq, Wk, Wv, Wo, trace=False):
    from concourse.bass_utils import run_bass_kernel_spmd

    nc = _get_nc()
    in_maps = _prep_inputs(q, k, v, Wq, Wk, Wv, Wo)
    res = run_bass_kernel_spmd(nc, in_maps, list(range(8)), trace=trace)
    out = np.zeros((B, S, D), np.float32)
    for c in range(8):
        out[c // 4] += np.asarray(res.results[c]["out"], np.float32)
    return out, res


def kernel(q, k, v, mask, Wq, Wk, Wv, Wo):
    out, _ = run_spmd(q, k, v, Wq, Wk, Wv, Wo, trace=False)
    return out
